# revision 23
# baseline (speedup 1.0000x reference)
"""AtomToTokenCrossAttn distributed Bass kernel for 8 TRN2 NeuronCores.

Sharding: the 16384 (B*N) token rows are split into 8 contiguous shards of
2048 rows (each core owns half of one batch's tokens). Because the atom
windows are deterministic/contiguous per token (starts = 8n), each core only
needs the contiguous atom slice covering its tokens' windows -- no
collectives needed.

v2 pipeline per core (bf16 matmuls, f32 accumulation):
  - host pre-shuffles a/s into partition-major layouts so every DMA is
    contiguous per partition
  - LayerNorm stats via ONE grouped bn_stats per block + small DVE combines;
    apply folded into ACT activation (gamma/beta pre-folded into weights)
  - a_n (atom-major) is kept and transposed once (PE) into aT (d-major).
    K is never materialized: scores = aT.T @ qw with qw = wk1^T-mixed
    queries (exact rewrite of (wk a_n)^T q4; the K bias ck cancels in
    softmax). V is never materialized: ctx = p @ a_n window (atom-major
    lhsT), then x = wv^T @ ctx with the V bias cv entering as +cv after
    division (sum p = 1).
  - ragged masking folded into the scores matmul as an additive -50 bias
    (Toeplitz step-matrix x host-built one-hot columns), PSUM-accumulated
    before the Q.K matmuls -- unchanged from v1.
  - softmax denominators: all-ones 128x128 matmul broadcasts column sums to
    every partition; reciprocal_approx_fast on [128,512]; one multiply per
    group normalizes the whole tile of exps.
  - wv-projection matmuls write token-major PSUM directly (per-head 32-row
    slices), so no extraction copies; gate sigmoid(G) and w_o applied per
    512-token chunk; token_mask applied on the host (commutes through w_o).
"""

import numpy as np
import ml_dtypes

import concourse.bass as bass
import concourse.mybir as mybir
import concourse.tile as tile
from concourse import bacc
from concourse.bass_utils import run_bass_kernel_spmd
from concourse.masks import make_identity

F32 = mybir.dt.float32
BF16 = mybir.dt.bfloat16
AOP = mybir.AluOpType
AFT = mybir.ActivationFunctionType
PSUM = bass.MemorySpace.PSUM

B, N, M = 4, 4096, 32768
D_TOK, D_ATOM, H, D_H = 512, 128, 4, 32
W_MAX = 16
LN_EPS = 1e-5
NC_CORES = 8
TOK = (B * N) // NC_CORES          # 2048 tokens per core
T = 16                             # tokens per attention tile
TILES = TOK // T                   # 128
COLS = TILES * H * T               # 8192 score columns (t, h, i)
SPAN_B = 8                         # spill atoms per tile (span 136 = 128+8)
NEG = -50.0
GRP = 8                            # tiles per attention group
N_GRP = TILES // GRP               # 16
GCOL = GRP * H * T                 # 512 columns per group
XG = 4                             # groups per 512-token output chunk

_cache = {}


def _ln_stats(nc, sp, st6, nch, inv_d, eps_sb, tag):
    """Combine grouped bn_stats halves -> per-chunk rstd and -mean*rstd.

    st6: [128, nch, 6] = (cnt,mean,cnt*var) of even / odd elements.
    Returns (rstd, nmr) tiles [128, nch] f32.
    """
    v = sp.tile([128, 32, 4], F32, tag=f"{tag}v", name=f"{tag}v")[:, :nch]
    # v0=m_e+m_o  v1=m_e-m_o  v2=cv_e+cv_o  v3=(m_e-m_o)^2
    nc.vector.tensor_tensor(v[:, :, 0], st6[:, :, 1], st6[:, :, 4], AOP.add)
    nc.vector.tensor_tensor(v[:, :, 1], st6[:, :, 1], st6[:, :, 4],
                            AOP.subtract)
    nc.vector.tensor_tensor(v[:, :, 2], st6[:, :, 2], st6[:, :, 5], AOP.add)
    nc.vector.tensor_tensor(v[:, :, 3], v[:, :, 1], v[:, :, 1], AOP.mult)
    var = sp.tile([128, 32], F32, tag=f"{tag}var", name=f"{tag}var")[:, :nch]
    nc.vector.tensor_scalar(var, v[:, :, 3], 0.25, None, AOP.mult)
    v2s = sp.tile([128, 32], F32, tag=f"{tag}v2", name=f"{tag}v2")[:, :nch]
    nc.vector.tensor_scalar(v2s, v[:, :, 2], inv_d, None, AOP.mult)
    nc.vector.tensor_tensor(var, var, v2s, AOP.add)
    rstd = sp.tile([128, 32], F32, tag=f"{tag}rs", name=f"{tag}rs")[:, :nch]
    nc.scalar.activation(rstd, var, AFT.Sqrt, bias=eps_sb)
    nc.vector.reciprocal(rstd, rstd)
    nmr = sp.tile([128, 32], F32, tag=f"{tag}nm", name=f"{tag}nm")[:, :nch]
    # nmr = -mean*rstd = -(0.5*msum)*rstd
    nc.vector.tensor_tensor(nmr, v[:, :, 0], rstd, AOP.mult)
    nc.vector.tensor_scalar(nmr, nmr, -0.5, None, AOP.mult)
    return rstd, nmr


def _build(nc, A_pad):
    CH_A = A_pad // 128
    NBLK = (CH_A + 15) // 16          # a blocks of <=16 chunks

    a8 = nc.declare_dram_parameter("a8", [128, CH_A * 128], BF16,
                                   isOutput=False)
    s8 = nc.declare_dram_parameter("s8", [128, 16 * 512], BF16,
                                   isOutput=False)
    rhs2 = nc.declare_dram_parameter("rhs2", [128, COLS], BF16, isOutput=False)
    rhs2b = nc.declare_dram_parameter("rhs2b", [9, COLS], BF16, isOutput=False)
    ubias = nc.declare_dram_parameter("ubias", [128, 128], BF16, isOutput=False)
    ubiasb = nc.declare_dram_parameter("ubiasb", [9, 8], BF16, isOutput=False)
    wq1 = nc.declare_dram_parameter("wq1", [512, 128], BF16, isOutput=False)
    wg1 = nc.declare_dram_parameter("wg1", [512, 128], BF16, isOutput=False)
    wk1t = nc.declare_dram_parameter("wk1t", [128, 512], BF16, isOutput=False)
    wv1 = nc.declare_dram_parameter("wv1", [128, 512], BF16, isOutput=False)
    wo = nc.declare_dram_parameter("wo", [128, 512], BF16, isOutput=False)
    cq = nc.declare_dram_parameter("cq", [128, 1], F32, isOutput=False)
    cg = nc.declare_dram_parameter("cg", [128, 1], F32, isOutput=False)
    cv = nc.declare_dram_parameter("cv", [128, 1], F32, isOutput=False)
    o_t = nc.declare_dram_parameter("o_t", [4, 128, TOK], F32, isOutput=True)

    a8v = a8[:, :].rearrange("p (c d) -> p c d", d=128)
    s8v = s8[:, :].rearrange("p (c d) -> p c d", d=512)

    with tile.TileContext(nc) as tc:
        with (
            tc.tile_pool(name="pp", bufs=1) as pp,
            tc.tile_pool(name="sp", bufs=4) as sp,
        ):
            # ---- constants / weights
            ident = pp.tile([128, 128], BF16)
            make_identity(nc, ident)
            ones_a = pp.tile([128, 128], BF16)
            nc.vector.memset(ones_a, 1.0)
            eps_sb = pp.tile([128, 1], F32)
            nc.vector.memset(eps_sb, LN_EPS)
            wq_sb = pp.tile([128, 4, 128], BF16)
            nc.sync.dma_start(wq_sb, wq1[:, :].rearrange("(c p) m -> p c m", p=128))
            wg_sb = pp.tile([128, 4, 128], BF16)
            nc.sync.dma_start(wg_sb, wg1[:, :].rearrange("(c p) m -> p c m", p=128))
            wk1t_sb = pp.tile([128, 4, 128], BF16)
            nc.sync.dma_start(wk1t_sb, wk1t[:, :].rearrange("k (h m) -> k h m", m=128))
            wv_sb = pp.tile([128, 4, 128], BF16)
            nc.sync.dma_start(wv_sb, wv1[:, :].rearrange("k (h m) -> k h m", m=128))
            wo_sb = pp.tile([128, 4, 128], BF16)
            nc.sync.dma_start(wo_sb, wo[:, :].rearrange("k (c m) -> k c m", m=128))
            cq_sb = pp.tile([128, 1], F32)
            nc.sync.dma_start(cq_sb, cq[:, :])
            cg_sb = pp.tile([128, 1], F32)
            nc.sync.dma_start(cg_sb, cg[:, :])
            cv_sb = pp.tile([128, 1], F32)
            nc.sync.dma_start(cv_sb, cv[:, :])
            ub_sb = pp.tile([128, 128], BF16)
            nc.sync.dma_start(ub_sb, ubias[:, :])
            ubb_sb = pp.tile([9, 8], BF16)
            nc.sync.dma_start(ubb_sb, ubiasb[:, :])
            # rhs2 DMAs are emitted later (after the a/s block loads) so the
            # big mask upload doesn't delay the first compute blocks
            rhs2_sb = pp.tile([128, COLS], BF16)
            rhs2b_sb = pp.tile([9, COLS], BF16)

            # persistent per-block activations
            a_n = [pp.tile([128, min(16, CH_A - b * 16), 128], BF16,
                           name=f"a_n{b}") for b in range(NBLK)]
            aT = [pp.tile([128, min(16, CH_A - b * 16) * 128], BF16,
                          name=f"aT{b}") for b in range(NBLK)]
            sT = [pp.tile([128, 4, 512], BF16, name=f"sT{b}") for b in range(4)]
            qt = [pp.tile([128, 512], BF16, name=f"qt{b}") for b in range(4)]
            gsig = [pp.tile([128, 512], BF16, name=f"gs{b}") for b in range(4)]
            # qw4[b][din, h, j] = per-head wk1-mixed queries, token b*512+j
            qw4 = [pp.tile([128, 4, 512], BF16, name=f"qw{b}")
                   for b in range(4)]

            def qw_tile(t):
                """[128, 4, 16] (h, i)-ordered query slice for tile t."""
                return qw4[t // 32][:, :, (t % 32) * 16:(t % 32) * 16 + 16]

            def a_chunk(c):
                return a_n[c // 16][:, c % 16, :]

            def aT_cols(c0, w):
                """aT slice covering atom cols [c0*128 .. c0*128+w)."""
                b = c0 // 16
                off = (c0 % 16) * 128
                return aT[b][:, off:off + w]

            # =================== s pipeline ===================
            with (
                tc.tile_pool(name="sdma", bufs=2) as sdma,
                tc.tile_pool(name="sw", bufs=2) as sw,
                tc.tile_pool(name="psT", bufs=2, space=PSUM) as psT,
                tc.tile_pool(name="psQ", bufs=2, space=PSUM) as psQ,
                tc.tile_pool(name="psW", bufs=2, space=PSUM) as psW,
            ):
                for b in range(4):
                    blk = sdma.tile([128, 4, 512], BF16, tag="sblk",
                                    name=f"sblk{b}")
                    nc.sync.dma_start(blk, s8v[:, b * 4:(b + 1) * 4, :])
                    st6 = sp.tile([128, 4, 6], F32, tag="sst6", name="sst6")
                    for c in range(4):
                        nc.vector.bn_stats(st6[:, c, :], blk[:, c, :])
                    rstd, nmr = _ln_stats(nc, sp, st6, 4, 1.0 / 512.0,
                                          eps_sb, "s")
                    s_nb = sw.tile([128, 4, 512], BF16, tag="snb", name="snb")
                    for c in range(4):
                        nc.scalar.activation(s_nb[:, c, :], blk[:, c, :],
                                             AFT.Identity,
                                             bias=nmr[:, c:c + 1],
                                             scale=rstd[:, c:c + 1])
                    for c in range(4):
                        ps_t = psT.tile([128, 512], BF16, tag="tbig",
                                        name="tbig")
                        for k in range(4):
                            nc.tensor.transpose(
                                ps_t[:, k * 128:(k + 1) * 128],
                                s_nb[:, c, k * 128:(k + 1) * 128], ident)
                        nc.vector.tensor_copy(
                            sT[b][:, :, c * 128:(c + 1) * 128],
                            ps_t[:, :].rearrange("p (k m) -> p k m", m=128))

                # Q (+cq) and sigmoid(G+cg), per 512-token chunk
                for b in range(4):
                    ps_q = psQ.tile([128, 512], F32, tag="big", name="big")
                    for k in range(4):
                        nc.tensor.matmul(ps_q, wq_sb[:, k, :], sT[b][:, k, :],
                                         start=(k == 0), stop=(k == 3))
                    nc.vector.tensor_scalar(qt[b], ps_q, cq_sb, None, AOP.add)
                    ps_g = psQ.tile([128, 512], F32, tag="big", name="big")
                    for k in range(4):
                        nc.tensor.matmul(ps_g, wg_sb[:, k, :], sT[b][:, k, :],
                                         start=(k == 0), stop=(k == 3))
                    nc.scalar.activation(gsig[b], ps_g, AFT.Sigmoid,
                                         bias=cg_sb)

                # qw = per-head wk1^T-mixed queries (h-major layout)
                for h in range(H):
                    for b in range(4):
                        qwp = psW.tile([128, 512], F32, tag="qwp",
                                       name="qwp")
                        nc.tensor.matmul(qwp, wk1t_sb[:, h, :], qt[b],
                                         start=True, stop=True)
                        nc.vector.tensor_copy(qw4[b][:, h, :], qwp)

            # =================== a pipeline ===================
            with (
                tc.tile_pool(name="adma", bufs=2) as adma,
                tc.tile_pool(name="psA", bufs=2, space=PSUM) as psA,
            ):
                for b in range(NBLK):
                    nch = min(16, CH_A - b * 16)
                    blk = adma.tile([128, 16, 128], BF16, tag="ablk",
                                    name=f"ablk{b}")[:, :nch]
                    nc.sync.dma_start(blk, a8v[:, b * 16:b * 16 + nch, :])
                    st6 = sp.tile([128, 16, 6], F32, tag="ast6",
                                  name="ast6")[:, :nch]
                    for c in range(nch):
                        nc.vector.bn_stats(st6[:, c, :], blk[:, c, :])
                    rstd, nmr = _ln_stats(nc, sp, st6, nch, 1.0 / 128.0,
                                          eps_sb, "a")
                    for c in range(nch):
                        # split the LN apply across ACT and DVE for balance
                        if c % 2 == 0:
                            nc.scalar.activation(a_n[b][:, c, :], blk[:, c, :],
                                                 AFT.Identity,
                                                 bias=nmr[:, c:c + 1],
                                                 scale=rstd[:, c:c + 1])
                        else:
                            nc.vector.tensor_scalar(a_n[b][:, c, :],
                                                    blk[:, c, :],
                                                    rstd[:, c:c + 1],
                                                    nmr[:, c:c + 1],
                                                    AOP.mult, AOP.add)
                    for q0 in range(0, nch, 4):
                        qn = min(4, nch - q0)
                        ps_t = psA.tile([128, 512], BF16, tag="tbig",
                                        name="tbig")
                        for k in range(qn):
                            nc.tensor.transpose(
                                ps_t[:, k * 128:(k + 1) * 128],
                                a_n[b][:, q0 + k, :], ident)
                        nc.vector.tensor_copy(
                            aT[b][:, q0 * 128:(q0 + qn) * 128],
                            ps_t[:, :qn * 128])

            nc.sync.dma_start(rhs2_sb, rhs2[:, :])
            nc.sync.dma_start(rhs2b_sb, rhs2b[:, :])

            # =================== attention ===================
            with (
                tc.tile_pool(name="ew", bufs=2) as ew,
                tc.tile_pool(name="psSA", bufs=2, space=PSUM) as psSA,
                tc.tile_pool(name="psSB", bufs=1, space=PSUM) as psSB,
                tc.tile_pool(name="psDN", bufs=1, space=PSUM) as psDN,
                tc.tile_pool(name="psCT", bufs=1, space=PSUM) as psCT,
                tc.tile_pool(name="psX", bufs=1, space=PSUM) as psX,
                tc.tile_pool(name="psO", bufs=1, space=PSUM) as psO,
            ):
                x_ps = None
                for g in range(N_GRP):
                    gsl = slice(g * GCOL, (g + 1) * GCOL)
                    sc_a = psSA.tile([128, GCOL], F32, tag="sc_a", name="sc_a")
                    sc_b = psSB.tile([8, GCOL], F32, tag="sc_b", name="sc_b")
                    nc.tensor.matmul(sc_a, ub_sb, rhs2_sb[:, gsl],
                                     start=True, stop=False)
                    nc.tensor.matmul(sc_b, ubb_sb[:, :], rhs2b_sb[:, gsl],
                                     start=True, stop=False)
                    for tt in range(GRP):
                        t = g * GRP + tt
                        csl = slice(tt * H * T, (tt + 1) * H * T)
                        nc.tensor.matmul(sc_a[:, csl], aT_cols(t, 128),
                                         qw_tile(t), start=False,
                                         stop=True, skip_group_check=True)
                        nc.tensor.matmul(sc_b[:, csl], aT_cols(t + 1, 8),
                                         qw_tile(t), start=False,
                                         stop=True, skip_group_check=True)
                    exp_a = ew.tile([128, GCOL], BF16, tag="exp_a",
                                    name="exp_a")
                    exp_b = ew.tile([8, GCOL], BF16, tag="exp_b", name="exp_b")
                    nc.scalar.activation(exp_a, sc_a, AFT.Exp)
                    nc.scalar.activation(exp_b, sc_b, AFT.Exp)
                    dnb = psDN.tile([128, GCOL], F32, tag="dnb", name="dnb")
                    nc.tensor.matmul(dnb, ones_a, exp_a,
                                     start=True, stop=False)
                    nc.tensor.matmul(dnb, ones_a[0:8, :], exp_b,
                                     start=False, stop=True,
                                     skip_group_check=True)
                    rec = ew.tile([128, GCOL], F32, tag="rec", name="rec")
                    nc.vector.reciprocal_approx_fast(rec, dnb)
                    p_a = ew.tile([128, GCOL], BF16, tag="p_a", name="p_a")
                    p_b = ew.tile([8, GCOL], BF16, tag="p_b", name="p_b")
                    nc.vector.tensor_tensor(p_a, exp_a, rec, AOP.mult)
                    nc.vector.tensor_tensor(p_b, exp_b, rec[0:8, :], AOP.mult)
                    ctx = psCT.tile([128, GCOL], F32, tag="ctx", name="ctx")
                    for tt in range(GRP):
                        t = g * GRP + tt
                        csl = slice(tt * H * T, (tt + 1) * H * T)
                        nc.tensor.matmul(ctx[:, csl], a_chunk(t),
                                         p_a[:, csl], start=True, stop=False,
                                         skip_group_check=True)
                        nc.tensor.matmul(ctx[:, csl], a_chunk(t + 1)[0:8, :],
                                         p_b[:, csl], start=False, stop=True,
                                         skip_group_check=True)
                    ctx_sb = ew.tile([128, GCOL], BF16, tag="ctx_sb",
                                     name="ctx_sb")
                    nc.scalar.activation(ctx_sb, ctx, AFT.Copy)
                    if g % XG == 0:
                        x_ps = psX.tile([128, 512], F32, tag="x_ps",
                                        name="x_ps")
                    xo = (g % XG) * 128
                    for h in range(H):
                        nc.tensor.matmul(
                            x_ps[:, xo:xo + 128],
                            wv_sb[:, h, :],
                            ctx_sb[:, :]
                            .rearrange("p (t c) -> p t c", c=H * T)
                            [:, :, h * T:(h + 1) * T],
                            start=(h == 0), stop=(h == 3),
                            skip_group_check=True)
                    if g % XG == XG - 1:
                        sub = g // XG
                        ssl = slice(sub * 512, (sub + 1) * 512)
                        xb = ew.tile([128, 512], BF16, tag="xb", name="xb")
                        nc.vector.tensor_scalar(xb, x_ps, cv_sb, None, AOP.add)
                        nc.vector.tensor_tensor(xb, xb, gsig[sub], AOP.mult)
                        for c in range(4):
                            ps_o = psO.tile([128, 512], F32, tag="ps_o",
                                            name="ps_o")
                            nc.tensor.matmul(ps_o, wo_sb[:, c, :], xb,
                                             start=True, stop=True)
                            ot_sb = ew.tile([128, 512], F32, tag="ot_sb",
                                            name="ot_sb")
                            nc.scalar.activation(ot_sb, ps_o, AFT.Copy)
                            nc.sync.dma_start(o_t[c, :, ssl], ot_sb)
    nc.compile()
    nc.finalize()
    return nc


def _prep(s, a, starts, counts, token_mask, w_q, w_k, w_v, w_g, w_o,
          ln_q_g, ln_q_b, ln_kv_g, ln_kv_b):
    bf = ml_dtypes.bfloat16
    sc = 1.0 / np.sqrt(np.float32(D_H))
    wq1 = ((ln_q_g[:, None] * w_q) * sc).astype(bf)
    wg1 = (ln_q_g[:, None] * w_g).astype(bf)
    # head-masked weight blocks (avoid partition-offset matmul operands):
    # wk1t[k, h*128+m] = wk1.T[k, m] if k in head-h block else 0
    wk1_t = np.asarray((ln_kv_g[:, None] * w_k).T, np.float32)  # [dout, din]
    wk1t = np.zeros((128, 4 * 128), np.float32)
    wv1_f = np.asarray(ln_kv_g[:, None] * w_v, np.float32)      # [din, dd]
    wv1 = np.zeros((128, 4 * 128), np.float32)
    for h in range(4):
        wk1t[h * 32:(h + 1) * 32, h * 128:(h + 1) * 128] = \
            wk1_t[h * 32:(h + 1) * 32, :]
        wv1[:, h * 128:(h + 1) * 128] = wv1_f * \
            (np.arange(128)[None, :] // 32 == h)
    wk1t = wk1t.astype(bf)
    wv1 = wv1.astype(bf)
    cq = ((ln_q_b @ w_q) * sc).astype(np.float32).reshape(128, 1)
    cg = (ln_q_b @ w_g).astype(np.float32).reshape(128, 1)
    cv = (ln_kv_b @ w_v).astype(np.float32).reshape(128, 1)

    jj = np.arange(128)
    ub = (NEG * (jj[None, :] > np.arange(128)[:, None])).astype(np.float32)
    ub[127, :] = NEG
    ubias = ub.astype(bf)
    jb = np.arange(8)
    ubb = (NEG * (jb[None, :] > np.arange(9)[:, None])).astype(np.float32)
    ubb[8, :] = NEG
    ubiasb = ubb.astype(bf)

    shards = []
    A_need = 128 * TILES + SPAN_B
    for c in range(NC_CORES):
        b, half = c // 2, c % 2
        n0 = half * TOK
        st = np.asarray(starts[b, n0:n0 + TOK], np.int64)
        ct = np.asarray(counts[b, n0:n0 + TOK], np.int64)
        lo = int(st.min())
        st_loc = st - lo
        end_loc = st_loc + ct
        bases = 128 * (np.arange(TOK) // T)
        off = st_loc - bases
        end = end_loc - bases
        assert off.min() >= 0 and off.max() <= 127, \
            f"window premise violated (off {off.min()}..{off.max()})"
        assert end.max() <= 128 + SPAN_B, \
            f"window premise violated (end max {end.max()})"
        shards.append((b, n0, lo, off, end))
        A_need = max(A_need, int(end_loc.max()))
    A_pad = ((A_need + 127) // 128) * 128

    k_tok = np.arange(TOK)
    t_idx = k_tok // T
    i_idx = k_tok % T

    in_maps = []
    for (b, n0, lo, off, end) in shards:
        a_sl = np.zeros((A_pad, 128), np.float32)
        hi = min(lo + A_pad, M)
        a_sl[:hi - lo] = np.asarray(a[b, lo:hi, :], np.float32)
        # partition-major: [128 p, CH_A c, 128 d], atom (c*128+p)
        a8 = a_sl.reshape(A_pad // 128, 128, 128).transpose(1, 0, 2) \
            .reshape(128, A_pad).astype(bf)
        s_sl = np.asarray(s[b, n0:n0 + TOK, :], np.float32)
        s8 = s_sl.reshape(16, 128, 512).transpose(1, 0, 2) \
            .reshape(128, 16 * 512).astype(bf)

        r2 = np.zeros((128, COLS), np.float32)
        r2b = np.zeros((9, COLS), np.float32)
        for h in range(H):
            cols = t_idx * (H * T) + h * T + i_idx
            m1 = off >= 1
            np.add.at(r2, (np.where(m1, off - 1, 0), cols),
                      np.where(m1, -1.0, 0.0))
            np.add.at(r2, (np.full(TOK, 127), cols), np.where(m1, 1.0, 0.0))
            m2 = end <= 127
            np.add.at(r2, (np.where(m2, end - 1, 0), cols),
                      np.where(m2, 1.0, 0.0))
            m3 = end <= 128
            np.add.at(r2b, (np.full(TOK, 8), cols), np.where(m3, 1.0, 0.0))
            m4 = end >= 129
            np.add.at(r2b, (np.where(m4, end - 129, 0), cols),
                      np.where(m4, 1.0, 0.0))
        in_maps.append({
            "a8": a8, "s8": s8,
            "rhs2": r2.astype(bf), "rhs2b": r2b.astype(bf),
            "ubias": ubias, "ubiasb": ubiasb,
            "wq1": wq1, "wg1": wg1, "wk1t": wk1t, "wv1": wv1,
            "wo": np.asarray(w_o, np.float32).astype(bf),
            "cq": cq, "cg": cg, "cv": cv,
        })
    return in_maps, A_pad


def kernel(s, a, token_atom_starts, token_atom_counts, token_mask,
           w_q, w_k, w_v, w_g, w_o, ln_q_g, ln_q_b, ln_kv_g, ln_kv_b,
           trace=False):
    args = [np.asarray(x) for x in
            (s, a, token_atom_starts, token_atom_counts, token_mask,
             w_q, w_k, w_v, w_g, w_o, ln_q_g, ln_q_b, ln_kv_g, ln_kv_b)]
    in_maps, A_pad = _prep(*args)
    if A_pad not in _cache:
        nc = bacc.Bacc(None, target_bir_lowering=False)
        _cache[A_pad] = _build(nc, A_pad)
    nc = _cache[A_pad]
    res = run_bass_kernel_spmd(nc, in_maps, list(range(NC_CORES)),
                               trace=trace)
    out = np.zeros((B, N, D_TOK), np.float32)
    for c in range(NC_CORES):
        b, half = c // 2, c % 2
        n0 = half * TOK
        ot = res.results[c]["o_t"]          # [4, 128, TOK]
        tm = np.asarray(args[4][b, n0:n0 + TOK], np.float32)
        out[b, n0:n0 + TOK, :] = ot.reshape(512, TOK).T * tm[:, None]
    kernel.last_exec_time_ns = res.exec_time_ns
    return out


# revision 24
# speedup vs baseline: 1.0909x; 1.0909x over previous
"""AtomToTokenCrossAttn distributed Bass kernel for 8 TRN2 NeuronCores.

Sharding: the 16384 (B*N) token rows are split into 8 contiguous shards of
2048 rows (each core owns half of one batch's tokens). Because the atom
windows are deterministic/contiguous per token (starts = 8n), each core only
needs the contiguous atom slice covering its tokens' windows -- no
collectives needed.

v2 pipeline per core (bf16 matmuls, f32 accumulation):
  - host pre-shuffles a/s into partition-major layouts so every DMA is
    contiguous per partition
  - LayerNorm stats via ONE grouped bn_stats per block + small DVE combines;
    apply folded into ACT activation (gamma/beta pre-folded into weights)
  - a_n (atom-major) is kept and transposed once (PE) into aT (d-major).
    K is never materialized: scores = aT.T @ qw with qw = wk1^T-mixed
    queries (exact rewrite of (wk a_n)^T q4; the K bias ck cancels in
    softmax). V is never materialized: ctx = p @ a_n window (atom-major
    lhsT), then x = wv^T @ ctx with the V bias cv entering as +cv after
    division (sum p = 1).
  - ragged masking folded into the scores matmul as an additive -50 bias
    (Toeplitz step-matrix x host-built one-hot columns), PSUM-accumulated
    before the Q.K matmuls -- unchanged from v1.
  - softmax denominators: all-ones 128x128 matmul broadcasts column sums to
    every partition; reciprocal_approx_fast on [128,512]; one multiply per
    group normalizes the whole tile of exps.
  - wv-projection matmuls write token-major PSUM directly (per-head 32-row
    slices), so no extraction copies; gate sigmoid(G) and w_o applied per
    512-token chunk; token_mask applied on the host (commutes through w_o).
"""

import numpy as np
import ml_dtypes

import concourse.bass as bass
import concourse.mybir as mybir
import concourse.tile as tile
from concourse import bacc
from concourse.bass_utils import run_bass_kernel_spmd
from concourse.masks import make_identity

F32 = mybir.dt.float32
BF16 = mybir.dt.bfloat16
AOP = mybir.AluOpType
AFT = mybir.ActivationFunctionType
PSUM = bass.MemorySpace.PSUM

B, N, M = 4, 4096, 32768
D_TOK, D_ATOM, H, D_H = 512, 128, 4, 32
W_MAX = 16
LN_EPS = 1e-5
NC_CORES = 8
TOK = (B * N) // NC_CORES          # 2048 tokens per core
T = 16                             # tokens per attention tile
TILES = TOK // T                   # 128
COLS = TILES * H * T               # 8192 score columns (t, h, i)
SPAN_B = 8                         # spill atoms per tile (span 136 = 128+8)
NEG = -50.0
GRP = 8                            # tiles per attention group
N_GRP = TILES // GRP               # 16
GCOL = GRP * H * T                 # 512 columns per group
XG = 4                             # groups per 512-token output chunk

_cache = {}


def _ln_stats(nc, sp, st6, nch, inv_d, eps_sb, tag):
    """Combine grouped bn_stats halves -> per-chunk rstd and -mean*rstd.

    st6: [128, nch, 6] = (cnt,mean,cnt*var) of even / odd elements.
    Returns (rstd, nmr) tiles [128, nch] f32.
    """
    v = sp.tile([128, 32, 4], F32, tag=f"{tag}v", name=f"{tag}v")[:, :nch]
    # v0=m_e+m_o  v1=m_e-m_o  v2=cv_e+cv_o  v3=(m_e-m_o)^2
    nc.vector.tensor_tensor(v[:, :, 0], st6[:, :, 1], st6[:, :, 4], AOP.add)
    nc.vector.tensor_tensor(v[:, :, 1], st6[:, :, 1], st6[:, :, 4],
                            AOP.subtract)
    nc.vector.tensor_tensor(v[:, :, 2], st6[:, :, 2], st6[:, :, 5], AOP.add)
    nc.vector.tensor_tensor(v[:, :, 3], v[:, :, 1], v[:, :, 1], AOP.mult)
    var = sp.tile([128, 32], F32, tag=f"{tag}var", name=f"{tag}var")[:, :nch]
    nc.vector.tensor_scalar(var, v[:, :, 3], 0.25, None, AOP.mult)
    v2s = sp.tile([128, 32], F32, tag=f"{tag}v2", name=f"{tag}v2")[:, :nch]
    nc.vector.tensor_scalar(v2s, v[:, :, 2], inv_d, None, AOP.mult)
    nc.vector.tensor_tensor(var, var, v2s, AOP.add)
    rstd = sp.tile([128, 32], F32, tag=f"{tag}rs", name=f"{tag}rs")[:, :nch]
    nc.scalar.activation(rstd, var, AFT.Sqrt, bias=eps_sb)
    nc.vector.reciprocal(rstd, rstd)
    nmr = sp.tile([128, 32], F32, tag=f"{tag}nm", name=f"{tag}nm")[:, :nch]
    # nmr = -mean*rstd = -(0.5*msum)*rstd
    nc.vector.tensor_tensor(nmr, v[:, :, 0], rstd, AOP.mult)
    nc.vector.tensor_scalar(nmr, nmr, -0.5, None, AOP.mult)
    return rstd, nmr


def _build(nc, A_pad):
    CH_A = A_pad // 128
    NBLK = (CH_A + 15) // 16          # a blocks of <=16 chunks

    a8 = nc.declare_dram_parameter("a8", [128, CH_A * 128], BF16,
                                   isOutput=False)
    s8 = nc.declare_dram_parameter("s8", [128, 16 * 512], BF16,
                                   isOutput=False)
    rhs2 = nc.declare_dram_parameter("rhs2", [128, COLS], BF16, isOutput=False)
    rhs2b = nc.declare_dram_parameter("rhs2b", [9, COLS], BF16, isOutput=False)
    ubias = nc.declare_dram_parameter("ubias", [128, 128], BF16, isOutput=False)
    ubiasb = nc.declare_dram_parameter("ubiasb", [9, 8], BF16, isOutput=False)
    wq1 = nc.declare_dram_parameter("wq1", [512, 128], BF16, isOutput=False)
    wg1 = nc.declare_dram_parameter("wg1", [512, 128], BF16, isOutput=False)
    wk1t = nc.declare_dram_parameter("wk1t", [128, 512], BF16, isOutput=False)
    wv1 = nc.declare_dram_parameter("wv1", [128, 512], BF16, isOutput=False)
    wo = nc.declare_dram_parameter("wo", [128, 512], BF16, isOutput=False)
    cq = nc.declare_dram_parameter("cq", [128, 1], F32, isOutput=False)
    cg = nc.declare_dram_parameter("cg", [128, 1], F32, isOutput=False)
    cv = nc.declare_dram_parameter("cv", [128, 1], F32, isOutput=False)
    o_t = nc.declare_dram_parameter("o_t", [4, 128, TOK], F32, isOutput=True)

    a8v = a8[:, :].rearrange("p (c d) -> p c d", d=128)
    s8v = s8[:, :].rearrange("p (c d) -> p c d", d=512)

    with tile.TileContext(nc) as tc:
        with (
            tc.tile_pool(name="pp", bufs=1) as pp,
            tc.tile_pool(name="sp", bufs=4) as sp,
        ):
            # ---- constants / weights
            ident = pp.tile([128, 128], BF16)
            make_identity(nc, ident)
            ones_a = pp.tile([128, 128], BF16)
            nc.vector.memset(ones_a, 1.0)
            eps_sb = pp.tile([128, 1], F32)
            nc.vector.memset(eps_sb, LN_EPS)
            wq_sb = pp.tile([128, 4, 128], BF16)
            nc.sync.dma_start(wq_sb, wq1[:, :].rearrange("(c p) m -> p c m", p=128))
            wg_sb = pp.tile([128, 4, 128], BF16)
            nc.sync.dma_start(wg_sb, wg1[:, :].rearrange("(c p) m -> p c m", p=128))
            wk1t_sb = pp.tile([128, 4, 128], BF16)
            nc.sync.dma_start(wk1t_sb, wk1t[:, :].rearrange("k (h m) -> k h m", m=128))
            wv_sb = pp.tile([128, 4, 128], BF16)
            nc.sync.dma_start(wv_sb, wv1[:, :].rearrange("k (h m) -> k h m", m=128))
            wo_sb = pp.tile([128, 4, 128], BF16)
            nc.sync.dma_start(wo_sb, wo[:, :].rearrange("k (c m) -> k c m", m=128))
            cq_sb = pp.tile([128, 1], F32)
            nc.sync.dma_start(cq_sb, cq[:, :])
            cg_sb = pp.tile([128, 1], F32)
            nc.sync.dma_start(cg_sb, cg[:, :])
            cv_sb = pp.tile([128, 1], F32)
            nc.sync.dma_start(cv_sb, cv[:, :])
            ub_sb = pp.tile([128, 128], BF16)
            nc.sync.dma_start(ub_sb, ubias[:, :])
            ubb_sb = pp.tile([9, 8], BF16)
            nc.sync.dma_start(ubb_sb, ubiasb[:, :])
            rhs2_sb = pp.tile([128, COLS], BF16)
            nc.sync.dma_start(rhs2_sb, rhs2[:, :])
            rhs2b_sb = pp.tile([9, COLS], BF16)
            nc.sync.dma_start(rhs2b_sb, rhs2b[:, :])

            # persistent per-block activations
            a_n = [pp.tile([128, min(16, CH_A - b * 16), 128], BF16,
                           name=f"a_n{b}") for b in range(NBLK)]
            aT = [pp.tile([128, min(16, CH_A - b * 16) * 128], BF16,
                          name=f"aT{b}") for b in range(NBLK)]
            sT = [pp.tile([128, 4, 512], BF16, name=f"sT{b}") for b in range(4)]
            qt = [pp.tile([128, 512], BF16, name=f"qt{b}") for b in range(4)]
            gsig = [pp.tile([128, 512], BF16, name=f"gs{b}") for b in range(4)]
            # qw4[b][din, h, j] = per-head wk1-mixed queries, token b*512+j
            qw4 = [pp.tile([128, 4, 512], BF16, name=f"qw{b}")
                   for b in range(4)]

            def qw_tile(t):
                """[128, 4, 16] (h, i)-ordered query slice for tile t."""
                return qw4[t // 32][:, :, (t % 32) * 16:(t % 32) * 16 + 16]

            def a_chunk(c):
                return a_n[c // 16][:, c % 16, :]

            def aT_cols(c0, w):
                """aT slice covering atom cols [c0*128 .. c0*128+w)."""
                b = c0 // 16
                off = (c0 % 16) * 128
                return aT[b][:, off:off + w]

            # =================== a pipeline ===================
            with (
                tc.tile_pool(name="adma", bufs=2) as adma,
                tc.tile_pool(name="psA", bufs=2, space=PSUM) as psA,
            ):
                for b in range(NBLK):
                    nch = min(16, CH_A - b * 16)
                    blk = adma.tile([128, 16, 128], BF16, tag="ablk",
                                    name=f"ablk{b}")[:, :nch]
                    nc.sync.dma_start(blk, a8v[:, b * 16:b * 16 + nch, :])
                    st6 = sp.tile([128, 16, 6], F32, tag="ast6",
                                  name="ast6")[:, :nch]
                    for c in range(nch):
                        nc.vector.bn_stats(st6[:, c, :], blk[:, c, :])
                    rstd, nmr = _ln_stats(nc, sp, st6, nch, 1.0 / 128.0,
                                          eps_sb, "a")
                    for c in range(nch):
                        nc.scalar.activation(a_n[b][:, c, :], blk[:, c, :],
                                             AFT.Identity,
                                             bias=nmr[:, c:c + 1],
                                             scale=rstd[:, c:c + 1])
                    for q0 in range(0, nch, 4):
                        qn = min(4, nch - q0)
                        ps_t = psA.tile([128, 512], BF16, tag="tbig",
                                        name="tbig")
                        for k in range(qn):
                            nc.tensor.transpose(
                                ps_t[:, k * 128:(k + 1) * 128],
                                a_n[b][:, q0 + k, :], ident)
                        nc.vector.tensor_copy(
                            aT[b][:, q0 * 128:(q0 + qn) * 128],
                            ps_t[:, :qn * 128])

            # =================== s pipeline ===================
            with (
                tc.tile_pool(name="sdma", bufs=2) as sdma,
                tc.tile_pool(name="sw", bufs=2) as sw,
                tc.tile_pool(name="psT", bufs=2, space=PSUM) as psT,
                tc.tile_pool(name="psQ", bufs=2, space=PSUM) as psQ,
                tc.tile_pool(name="psW", bufs=2, space=PSUM) as psW,
            ):
                for b in range(4):
                    blk = sdma.tile([128, 4, 512], BF16, tag="sblk",
                                    name=f"sblk{b}")
                    nc.sync.dma_start(blk, s8v[:, b * 4:(b + 1) * 4, :])
                    st6 = sp.tile([128, 4, 6], F32, tag="sst6", name="sst6")
                    for c in range(4):
                        nc.vector.bn_stats(st6[:, c, :], blk[:, c, :])
                    rstd, nmr = _ln_stats(nc, sp, st6, 4, 1.0 / 512.0,
                                          eps_sb, "s")
                    s_nb = sw.tile([128, 4, 512], BF16, tag="snb", name="snb")
                    for c in range(4):
                        nc.scalar.activation(s_nb[:, c, :], blk[:, c, :],
                                             AFT.Identity,
                                             bias=nmr[:, c:c + 1],
                                             scale=rstd[:, c:c + 1])
                    for c in range(4):
                        ps_t = psT.tile([128, 512], BF16, tag="tbig",
                                        name="tbig")
                        for k in range(4):
                            nc.tensor.transpose(
                                ps_t[:, k * 128:(k + 1) * 128],
                                s_nb[:, c, k * 128:(k + 1) * 128], ident)
                        nc.vector.tensor_copy(
                            sT[b][:, :, c * 128:(c + 1) * 128],
                            ps_t[:, :].rearrange("p (k m) -> p k m", m=128))

                # Q (+cq) and sigmoid(G+cg), per 512-token chunk
                for b in range(4):
                    ps_q = psQ.tile([128, 512], F32, tag="big", name="big")
                    for k in range(4):
                        nc.tensor.matmul(ps_q, wq_sb[:, k, :], sT[b][:, k, :],
                                         start=(k == 0), stop=(k == 3))
                    nc.vector.tensor_scalar(qt[b], ps_q, cq_sb, None, AOP.add)
                    ps_g = psQ.tile([128, 512], F32, tag="big", name="big")
                    for k in range(4):
                        nc.tensor.matmul(ps_g, wg_sb[:, k, :], sT[b][:, k, :],
                                         start=(k == 0), stop=(k == 3))
                    nc.scalar.activation(gsig[b], ps_g, AFT.Sigmoid,
                                         bias=cg_sb)

                # qw = per-head wk1^T-mixed queries (h-major layout)
                for h in range(H):
                    for b in range(4):
                        qwp = psW.tile([128, 512], F32, tag="qwp",
                                       name="qwp")
                        nc.tensor.matmul(qwp, wk1t_sb[:, h, :], qt[b],
                                         start=True, stop=True)
                        nc.vector.tensor_copy(qw4[b][:, h, :], qwp)

            # =================== attention ===================
            with (
                tc.tile_pool(name="ew", bufs=2) as ew,
                tc.tile_pool(name="psSA", bufs=2, space=PSUM) as psSA,
                tc.tile_pool(name="psSB", bufs=1, space=PSUM) as psSB,
                tc.tile_pool(name="psDN", bufs=1, space=PSUM) as psDN,
                tc.tile_pool(name="psCT", bufs=1, space=PSUM) as psCT,
                tc.tile_pool(name="psX", bufs=2, space=PSUM) as psX,
                tc.tile_pool(name="psO", bufs=1, space=PSUM) as psO,
            ):
                x_ps = None
                for g in range(N_GRP):
                    gsl = slice(g * GCOL, (g + 1) * GCOL)
                    sc_a = psSA.tile([128, GCOL], F32, tag="sc_a", name="sc_a")
                    sc_b = psSB.tile([8, GCOL], F32, tag="sc_b", name="sc_b")
                    nc.tensor.matmul(sc_a, ub_sb, rhs2_sb[:, gsl],
                                     start=True, stop=False)
                    nc.tensor.matmul(sc_b, ubb_sb[:, :], rhs2b_sb[:, gsl],
                                     start=True, stop=False)
                    for tt in range(GRP):
                        t = g * GRP + tt
                        csl = slice(tt * H * T, (tt + 1) * H * T)
                        nc.tensor.matmul(sc_a[:, csl], aT_cols(t, 128),
                                         qw_tile(t), start=False,
                                         stop=True, skip_group_check=True)
                        nc.tensor.matmul(sc_b[:, csl], aT_cols(t + 1, 8),
                                         qw_tile(t), start=False,
                                         stop=True, skip_group_check=True)
                    exp_a = ew.tile([128, GCOL], BF16, tag="exp_a",
                                    name="exp_a")
                    exp_b = ew.tile([8, GCOL], BF16, tag="exp_b", name="exp_b")
                    nc.scalar.activation(exp_a, sc_a, AFT.Exp)
                    nc.scalar.activation(exp_b, sc_b, AFT.Exp)
                    dnb = psDN.tile([128, GCOL], F32, tag="dnb", name="dnb")
                    nc.tensor.matmul(dnb, ones_a, exp_a,
                                     start=True, stop=False)
                    nc.tensor.matmul(dnb, ones_a[0:8, :], exp_b,
                                     start=False, stop=True,
                                     skip_group_check=True)
                    rec = ew.tile([128, GCOL], F32, tag="rec", name="rec")
                    nc.vector.reciprocal_approx_fast(rec, dnb)
                    p_a = ew.tile([128, GCOL], BF16, tag="p_a", name="p_a")
                    p_b = ew.tile([8, GCOL], BF16, tag="p_b", name="p_b")
                    nc.vector.tensor_tensor(p_a, exp_a, rec, AOP.mult)
                    nc.vector.tensor_tensor(p_b, exp_b, rec[0:8, :], AOP.mult)
                    ctx = psCT.tile([128, GCOL], F32, tag="ctx", name="ctx")
                    for tt in range(GRP):
                        t = g * GRP + tt
                        csl = slice(tt * H * T, (tt + 1) * H * T)
                        nc.tensor.matmul(ctx[:, csl], a_chunk(t),
                                         p_a[:, csl], start=True, stop=False,
                                         skip_group_check=True)
                        nc.tensor.matmul(ctx[:, csl], a_chunk(t + 1)[0:8, :],
                                         p_b[:, csl], start=False, stop=True,
                                         skip_group_check=True)
                    ctx_sb = ew.tile([128, GCOL], BF16, tag="ctx_sb",
                                     name="ctx_sb")
                    nc.scalar.activation(ctx_sb, ctx, AFT.Copy)
                    if g % XG == 0:
                        x_ps = psX.tile([128, 512], F32, tag="x_ps",
                                        name="x_ps")
                    xo = (g % XG) * 128
                    for h in range(H):
                        nc.tensor.matmul(
                            x_ps[:, xo:xo + 128],
                            wv_sb[:, h, :],
                            ctx_sb[:, :]
                            .rearrange("p (t c) -> p t c", c=H * T)
                            [:, :, h * T:(h + 1) * T],
                            start=(h == 0), stop=(h == 3),
                            skip_group_check=True)
                    if g % XG == XG - 1:
                        sub = g // XG
                        ssl = slice(sub * 512, (sub + 1) * 512)
                        xb = ew.tile([128, 512], BF16, tag="xb", name="xb")
                        nc.vector.tensor_scalar(xb, x_ps, cv_sb, None, AOP.add)
                        nc.vector.tensor_tensor(xb, xb, gsig[sub], AOP.mult)
                        for c in range(4):
                            ps_o = psO.tile([128, 512], F32, tag="ps_o",
                                            name="ps_o")
                            nc.tensor.matmul(ps_o, wo_sb[:, c, :], xb,
                                             start=True, stop=True)
                            ot_sb = ew.tile([128, 512], F32, tag="ot_sb",
                                            name="ot_sb")
                            nc.scalar.activation(ot_sb, ps_o, AFT.Copy)
                            nc.sync.dma_start(o_t[c, :, ssl], ot_sb)
    nc.compile()
    nc.finalize()
    return nc


def _prep(s, a, starts, counts, token_mask, w_q, w_k, w_v, w_g, w_o,
          ln_q_g, ln_q_b, ln_kv_g, ln_kv_b):
    bf = ml_dtypes.bfloat16
    sc = 1.0 / np.sqrt(np.float32(D_H))
    wq1 = ((ln_q_g[:, None] * w_q) * sc).astype(bf)
    wg1 = (ln_q_g[:, None] * w_g).astype(bf)
    # head-masked weight blocks (avoid partition-offset matmul operands):
    # wk1t[k, h*128+m] = wk1.T[k, m] if k in head-h block else 0
    wk1_t = np.asarray((ln_kv_g[:, None] * w_k).T, np.float32)  # [dout, din]
    wk1t = np.zeros((128, 4 * 128), np.float32)
    wv1_f = np.asarray(ln_kv_g[:, None] * w_v, np.float32)      # [din, dd]
    wv1 = np.zeros((128, 4 * 128), np.float32)
    for h in range(4):
        wk1t[h * 32:(h + 1) * 32, h * 128:(h + 1) * 128] = \
            wk1_t[h * 32:(h + 1) * 32, :]
        wv1[:, h * 128:(h + 1) * 128] = wv1_f * \
            (np.arange(128)[None, :] // 32 == h)
    wk1t = wk1t.astype(bf)
    wv1 = wv1.astype(bf)
    cq = ((ln_q_b @ w_q) * sc).astype(np.float32).reshape(128, 1)
    cg = (ln_q_b @ w_g).astype(np.float32).reshape(128, 1)
    cv = (ln_kv_b @ w_v).astype(np.float32).reshape(128, 1)

    jj = np.arange(128)
    ub = (NEG * (jj[None, :] > np.arange(128)[:, None])).astype(np.float32)
    ub[127, :] = NEG
    ubias = ub.astype(bf)
    jb = np.arange(8)
    ubb = (NEG * (jb[None, :] > np.arange(9)[:, None])).astype(np.float32)
    ubb[8, :] = NEG
    ubiasb = ubb.astype(bf)

    shards = []
    A_need = 128 * TILES + SPAN_B
    for c in range(NC_CORES):
        b, half = c // 2, c % 2
        n0 = half * TOK
        st = np.asarray(starts[b, n0:n0 + TOK], np.int64)
        ct = np.asarray(counts[b, n0:n0 + TOK], np.int64)
        lo = int(st.min())
        st_loc = st - lo
        end_loc = st_loc + ct
        bases = 128 * (np.arange(TOK) // T)
        off = st_loc - bases
        end = end_loc - bases
        assert off.min() >= 0 and off.max() <= 127, \
            f"window premise violated (off {off.min()}..{off.max()})"
        assert end.max() <= 128 + SPAN_B, \
            f"window premise violated (end max {end.max()})"
        shards.append((b, n0, lo, off, end))
        A_need = max(A_need, int(end_loc.max()))
    A_pad = ((A_need + 127) // 128) * 128

    k_tok = np.arange(TOK)
    t_idx = k_tok // T
    i_idx = k_tok % T

    in_maps = []
    for (b, n0, lo, off, end) in shards:
        a_sl = np.zeros((A_pad, 128), np.float32)
        hi = min(lo + A_pad, M)
        a_sl[:hi - lo] = np.asarray(a[b, lo:hi, :], np.float32)
        # partition-major: [128 p, CH_A c, 128 d], atom (c*128+p)
        a8 = a_sl.reshape(A_pad // 128, 128, 128).transpose(1, 0, 2) \
            .reshape(128, A_pad).astype(bf)
        s_sl = np.asarray(s[b, n0:n0 + TOK, :], np.float32)
        s8 = s_sl.reshape(16, 128, 512).transpose(1, 0, 2) \
            .reshape(128, 16 * 512).astype(bf)

        r2 = np.zeros((128, COLS), np.float32)
        r2b = np.zeros((9, COLS), np.float32)
        for h in range(H):
            cols = t_idx * (H * T) + h * T + i_idx
            m1 = off >= 1
            np.add.at(r2, (np.where(m1, off - 1, 0), cols),
                      np.where(m1, -1.0, 0.0))
            np.add.at(r2, (np.full(TOK, 127), cols), np.where(m1, 1.0, 0.0))
            m2 = end <= 127
            np.add.at(r2, (np.where(m2, end - 1, 0), cols),
                      np.where(m2, 1.0, 0.0))
            m3 = end <= 128
            np.add.at(r2b, (np.full(TOK, 8), cols), np.where(m3, 1.0, 0.0))
            m4 = end >= 129
            np.add.at(r2b, (np.where(m4, end - 129, 0), cols),
                      np.where(m4, 1.0, 0.0))
        in_maps.append({
            "a8": a8, "s8": s8,
            "rhs2": r2.astype(bf), "rhs2b": r2b.astype(bf),
            "ubias": ubias, "ubiasb": ubiasb,
            "wq1": wq1, "wg1": wg1, "wk1t": wk1t, "wv1": wv1,
            "wo": np.asarray(w_o, np.float32).astype(bf),
            "cq": cq, "cg": cg, "cv": cv,
        })
    return in_maps, A_pad


def kernel(s, a, token_atom_starts, token_atom_counts, token_mask,
           w_q, w_k, w_v, w_g, w_o, ln_q_g, ln_q_b, ln_kv_g, ln_kv_b,
           trace=False):
    args = [np.asarray(x) for x in
            (s, a, token_atom_starts, token_atom_counts, token_mask,
             w_q, w_k, w_v, w_g, w_o, ln_q_g, ln_q_b, ln_kv_g, ln_kv_b)]
    in_maps, A_pad = _prep(*args)
    if A_pad not in _cache:
        nc = bacc.Bacc(None, target_bir_lowering=False)
        _cache[A_pad] = _build(nc, A_pad)
    nc = _cache[A_pad]
    res = run_bass_kernel_spmd(nc, in_maps, list(range(NC_CORES)),
                               trace=trace)
    out = np.zeros((B, N, D_TOK), np.float32)
    for c in range(NC_CORES):
        b, half = c // 2, c % 2
        n0 = half * TOK
        ot = res.results[c]["o_t"]          # [4, 128, TOK]
        tm = np.asarray(args[4][b, n0:n0 + TOK], np.float32)
        out[b, n0:n0 + TOK, :] = ot.reshape(512, TOK).T * tm[:, None]
    kernel.last_exec_time_ns = res.exec_time_ns
    return out


# revision 25
# speedup vs baseline: 1.1456x; 1.0501x over previous
"""AtomToTokenCrossAttn distributed Bass kernel for 8 TRN2 NeuronCores.

Sharding: the 16384 (B*N) token rows are split into 8 contiguous shards of
2048 rows (each core owns half of one batch's tokens). Because the atom
windows are deterministic/contiguous per token (starts = 8n), each core only
needs the contiguous atom slice covering its tokens' windows -- no
collectives needed.

v2 pipeline per core (bf16 matmuls, f32 accumulation):
  - host pre-shuffles a/s into partition-major layouts so every DMA is
    contiguous per partition
  - LayerNorm stats via ONE grouped bn_stats per block + small DVE combines;
    apply folded into ACT activation (gamma/beta pre-folded into weights)
  - a_n (atom-major) is kept and transposed once (PE) into aT (d-major).
    K is never materialized: scores = aT.T @ qw with qw = wk1^T-mixed
    queries (exact rewrite of (wk a_n)^T q4; the K bias ck cancels in
    softmax). V is never materialized: ctx = p @ a_n window (atom-major
    lhsT), then x = wv^T @ ctx with the V bias cv entering as +cv after
    division (sum p = 1).
  - ragged masking folded into the scores matmul as an additive -50 bias
    (Toeplitz step-matrix x host-built one-hot columns), PSUM-accumulated
    before the Q.K matmuls -- unchanged from v1.
  - softmax denominators: all-ones 128x128 matmul broadcasts column sums to
    every partition; reciprocal_approx_fast on [128,512]; one multiply per
    group normalizes the whole tile of exps.
  - wv-projection matmuls write token-major PSUM directly (per-head 32-row
    slices), so no extraction copies; gate sigmoid(G) and w_o applied per
    512-token chunk; token_mask applied on the host (commutes through w_o).
"""

import numpy as np
import ml_dtypes

import concourse.bass as bass
import concourse.mybir as mybir
import concourse.tile as tile
from concourse import bacc
from concourse.bass_utils import run_bass_kernel_spmd
from concourse.masks import make_identity

F32 = mybir.dt.float32
BF16 = mybir.dt.bfloat16
AOP = mybir.AluOpType
AFT = mybir.ActivationFunctionType
PSUM = bass.MemorySpace.PSUM

B, N, M = 4, 4096, 32768
D_TOK, D_ATOM, H, D_H = 512, 128, 4, 32
W_MAX = 16
LN_EPS = 1e-5
NC_CORES = 8
TOK = (B * N) // NC_CORES          # 2048 tokens per core
T = 16                             # tokens per attention tile
TILES = TOK // T                   # 128
COLS = TILES * H * T               # 8192 score columns (t, h, i)
SPAN_B = 8                         # spill atoms per tile (span 136 = 128+8)
NEG = -50.0
GRP = 8                            # tiles per attention group
N_GRP = TILES // GRP               # 16
GCOL = GRP * H * T                 # 512 columns per group
XG = 4                             # groups per 512-token output chunk

_cache = {}


def _ln_stats(nc, sp, st6, nch, inv_d, eps_sb, tag):
    """Combine grouped bn_stats halves -> per-chunk rstd and -mean*rstd.

    st6: [128, nch, 6] = (cnt,mean,cnt*var) of even / odd elements.
    Returns (rstd, nmr) tiles [128, nch] f32.
    """
    v = sp.tile([128, 32, 4], F32, tag=f"{tag}v", name=f"{tag}v")[:, :nch]
    # v0=m_e+m_o  v1=m_e-m_o  v2=cv_e+cv_o  v3=(m_e-m_o)^2
    nc.vector.tensor_tensor(v[:, :, 0], st6[:, :, 1], st6[:, :, 4], AOP.add)
    nc.vector.tensor_tensor(v[:, :, 1], st6[:, :, 1], st6[:, :, 4],
                            AOP.subtract)
    nc.vector.tensor_tensor(v[:, :, 2], st6[:, :, 2], st6[:, :, 5], AOP.add)
    nc.vector.tensor_tensor(v[:, :, 3], v[:, :, 1], v[:, :, 1], AOP.mult)
    var = sp.tile([128, 32], F32, tag=f"{tag}var", name=f"{tag}var")[:, :nch]
    nc.vector.tensor_scalar(var, v[:, :, 3], 0.25, None, AOP.mult)
    v2s = sp.tile([128, 32], F32, tag=f"{tag}v2", name=f"{tag}v2")[:, :nch]
    nc.vector.tensor_scalar(v2s, v[:, :, 2], inv_d, None, AOP.mult)
    nc.vector.tensor_tensor(var, var, v2s, AOP.add)
    rstd = sp.tile([128, 32], F32, tag=f"{tag}rs", name=f"{tag}rs")[:, :nch]
    nc.scalar.activation(rstd, var, AFT.Sqrt, bias=eps_sb)
    nc.vector.reciprocal(rstd, rstd)
    nmr = sp.tile([128, 32], F32, tag=f"{tag}nm", name=f"{tag}nm")[:, :nch]
    # nmr = -mean*rstd = -(0.5*msum)*rstd
    nc.vector.tensor_tensor(nmr, v[:, :, 0], rstd, AOP.mult)
    nc.vector.tensor_scalar(nmr, nmr, -0.5, None, AOP.mult)
    return rstd, nmr


def _build(nc, A_pad):
    CH_A = A_pad // 128
    NBLK = (CH_A + 15) // 16          # a blocks of <=16 chunks

    a8 = nc.declare_dram_parameter("a8", [128, CH_A * 128], BF16,
                                   isOutput=False)
    s8 = nc.declare_dram_parameter("s8", [128, 16 * 512], BF16,
                                   isOutput=False)
    rhs2 = nc.declare_dram_parameter("rhs2", [128, COLS], BF16, isOutput=False)
    rhs2b = nc.declare_dram_parameter("rhs2b", [9, COLS], BF16, isOutput=False)
    ubias = nc.declare_dram_parameter("ubias", [128, 128], BF16, isOutput=False)
    ubiasb = nc.declare_dram_parameter("ubiasb", [9, 8], BF16, isOutput=False)
    wq1 = nc.declare_dram_parameter("wq1", [512, 128], BF16, isOutput=False)
    wg1 = nc.declare_dram_parameter("wg1", [512, 128], BF16, isOutput=False)
    wk1t = nc.declare_dram_parameter("wk1t", [128, 512], BF16, isOutput=False)
    wv1 = nc.declare_dram_parameter("wv1", [128, 512], BF16, isOutput=False)
    wo = nc.declare_dram_parameter("wo", [128, 512], BF16, isOutput=False)
    cq = nc.declare_dram_parameter("cq", [128, 1], F32, isOutput=False)
    cg = nc.declare_dram_parameter("cg", [128, 1], F32, isOutput=False)
    cv = nc.declare_dram_parameter("cv", [128, 1], F32, isOutput=False)
    o_t = nc.declare_dram_parameter("o_t", [4, 128, TOK], F32, isOutput=True)

    a8v = a8[:, :].rearrange("p (c d) -> p c d", d=128)
    s8v = s8[:, :].rearrange("p (c d) -> p c d", d=512)

    with tile.TileContext(nc) as tc:
        with (
            tc.tile_pool(name="pp", bufs=1) as pp,
            tc.tile_pool(name="sp", bufs=4) as sp,
        ):
            # ---- constants / weights
            ident = pp.tile([128, 128], BF16)
            make_identity(nc, ident)
            ones_a = pp.tile([128, 128], BF16)
            nc.vector.memset(ones_a, 1.0)
            eps_sb = pp.tile([128, 1], F32)
            nc.vector.memset(eps_sb, LN_EPS)
            wq_sb = pp.tile([128, 4, 128], BF16)
            nc.gpsimd.dma_start(wq_sb, wq1[:, :].rearrange("(c p) m -> p c m", p=128))
            wg_sb = pp.tile([128, 4, 128], BF16)
            nc.gpsimd.dma_start(wg_sb, wg1[:, :].rearrange("(c p) m -> p c m", p=128))
            wk1t_sb = pp.tile([128, 4, 128], BF16)
            nc.gpsimd.dma_start(wk1t_sb, wk1t[:, :].rearrange("k (h m) -> k h m", m=128))
            wv_sb = pp.tile([128, 4, 128], BF16)
            nc.gpsimd.dma_start(wv_sb, wv1[:, :].rearrange("k (h m) -> k h m", m=128))
            wo_sb = pp.tile([128, 4, 128], BF16)
            nc.gpsimd.dma_start(wo_sb, wo[:, :].rearrange("k (c m) -> k c m", m=128))
            cq_sb = pp.tile([128, 1], F32)
            nc.gpsimd.dma_start(cq_sb, cq[:, :])
            cg_sb = pp.tile([128, 1], F32)
            nc.gpsimd.dma_start(cg_sb, cg[:, :])
            cv_sb = pp.tile([128, 1], F32)
            nc.gpsimd.dma_start(cv_sb, cv[:, :])
            ub_sb = pp.tile([128, 128], BF16)
            nc.gpsimd.dma_start(ub_sb, ubias[:, :])
            ubb_sb = pp.tile([9, 8], BF16)
            nc.gpsimd.dma_start(ubb_sb, ubiasb[:, :])
            rhs2_sb = pp.tile([128, COLS], BF16)
            nc.gpsimd.dma_start(rhs2_sb, rhs2[:, :])
            rhs2b_sb = pp.tile([9, COLS], BF16)
            nc.gpsimd.dma_start(rhs2b_sb, rhs2b[:, :])

            # persistent per-block activations
            a_n = [pp.tile([128, min(16, CH_A - b * 16), 128], BF16,
                           name=f"a_n{b}") for b in range(NBLK)]
            aT = [pp.tile([128, min(16, CH_A - b * 16) * 128], BF16,
                          name=f"aT{b}") for b in range(NBLK)]
            sT = [pp.tile([128, 4, 512], BF16, name=f"sT{b}") for b in range(4)]
            qt = [pp.tile([128, 512], BF16, name=f"qt{b}") for b in range(4)]
            gsig = [pp.tile([128, 512], BF16, name=f"gs{b}") for b in range(4)]
            # qw4[b][din, h, j] = per-head wk1-mixed queries, token b*512+j
            qw4 = [pp.tile([128, 4, 512], BF16, name=f"qw{b}")
                   for b in range(4)]

            def qw_tile(t):
                """[128, 4, 16] (h, i)-ordered query slice for tile t."""
                return qw4[t // 32][:, :, (t % 32) * 16:(t % 32) * 16 + 16]

            def a_chunk(c):
                return a_n[c // 16][:, c % 16, :]

            def aT_cols(c0, w):
                """aT slice covering atom cols [c0*128 .. c0*128+w)."""
                b = c0 // 16
                off = (c0 % 16) * 128
                return aT[b][:, off:off + w]

            # =================== a pipeline ===================
            with (
                tc.tile_pool(name="adma", bufs=2) as adma,
                tc.tile_pool(name="psA", bufs=2, space=PSUM) as psA,
            ):
                for b in range(NBLK):
                    nch = min(16, CH_A - b * 16)
                    blk = adma.tile([128, 16, 128], BF16, tag="ablk",
                                    name=f"ablk{b}")[:, :nch]
                    nc.sync.dma_start(blk, a8v[:, b * 16:b * 16 + nch, :])
                    st6 = sp.tile([128, 16, 6], F32, tag="ast6",
                                  name="ast6")[:, :nch]
                    for c in range(nch):
                        nc.vector.bn_stats(st6[:, c, :], blk[:, c, :])
                    rstd, nmr = _ln_stats(nc, sp, st6, nch, 1.0 / 128.0,
                                          eps_sb, "a")
                    for c in range(nch):
                        nc.scalar.activation(a_n[b][:, c, :], blk[:, c, :],
                                             AFT.Identity,
                                             bias=nmr[:, c:c + 1],
                                             scale=rstd[:, c:c + 1])
                    for q0 in range(0, nch, 4):
                        qn = min(4, nch - q0)
                        ps_t = psA.tile([128, 512], BF16, tag="tbig",
                                        name="tbig")
                        for k in range(qn):
                            nc.tensor.transpose(
                                ps_t[:, k * 128:(k + 1) * 128],
                                a_n[b][:, q0 + k, :], ident)
                        nc.vector.tensor_copy(
                            aT[b][:, q0 * 128:(q0 + qn) * 128],
                            ps_t[:, :qn * 128])

            # =================== s pipeline ===================
            with (
                tc.tile_pool(name="sdma", bufs=2) as sdma,
                tc.tile_pool(name="sw", bufs=2) as sw,
                tc.tile_pool(name="psT", bufs=2, space=PSUM) as psT,
                tc.tile_pool(name="psQ", bufs=2, space=PSUM) as psQ,
                tc.tile_pool(name="psW", bufs=2, space=PSUM) as psW,
            ):
                for b in range(4):
                    blk = sdma.tile([128, 4, 512], BF16, tag="sblk",
                                    name=f"sblk{b}")
                    nc.sync.dma_start(blk, s8v[:, b * 4:(b + 1) * 4, :])
                    st6 = sp.tile([128, 4, 6], F32, tag="sst6", name="sst6")
                    for c in range(4):
                        nc.vector.bn_stats(st6[:, c, :], blk[:, c, :])
                    rstd, nmr = _ln_stats(nc, sp, st6, 4, 1.0 / 512.0,
                                          eps_sb, "s")
                    s_nb = sw.tile([128, 4, 512], BF16, tag="snb", name="snb")
                    for c in range(4):
                        nc.scalar.activation(s_nb[:, c, :], blk[:, c, :],
                                             AFT.Identity,
                                             bias=nmr[:, c:c + 1],
                                             scale=rstd[:, c:c + 1])
                    for c in range(4):
                        ps_t = psT.tile([128, 512], BF16, tag="tbig",
                                        name="tbig")
                        for k in range(4):
                            nc.tensor.transpose(
                                ps_t[:, k * 128:(k + 1) * 128],
                                s_nb[:, c, k * 128:(k + 1) * 128], ident)
                        nc.vector.tensor_copy(
                            sT[b][:, :, c * 128:(c + 1) * 128],
                            ps_t[:, :].rearrange("p (k m) -> p k m", m=128))

                # Q (+cq) and sigmoid(G+cg), per 512-token chunk
                for b in range(4):
                    ps_q = psQ.tile([128, 512], F32, tag="big", name="big")
                    for k in range(4):
                        nc.tensor.matmul(ps_q, wq_sb[:, k, :], sT[b][:, k, :],
                                         start=(k == 0), stop=(k == 3))
                    nc.vector.tensor_scalar(qt[b], ps_q, cq_sb, None, AOP.add)
                    ps_g = psQ.tile([128, 512], F32, tag="big", name="big")
                    for k in range(4):
                        nc.tensor.matmul(ps_g, wg_sb[:, k, :], sT[b][:, k, :],
                                         start=(k == 0), stop=(k == 3))
                    nc.scalar.activation(gsig[b], ps_g, AFT.Sigmoid,
                                         bias=cg_sb)

                # qw = per-head wk1^T-mixed queries (h-major layout)
                for h in range(H):
                    for b in range(4):
                        qwp = psW.tile([128, 512], F32, tag="qwp",
                                       name="qwp")
                        nc.tensor.matmul(qwp, wk1t_sb[:, h, :], qt[b],
                                         start=True, stop=True)
                        nc.vector.tensor_copy(qw4[b][:, h, :], qwp)

            # =================== attention ===================
            with (
                tc.tile_pool(name="ew", bufs=2) as ew,
                tc.tile_pool(name="psSA", bufs=2, space=PSUM) as psSA,
                tc.tile_pool(name="psSB", bufs=1, space=PSUM) as psSB,
                tc.tile_pool(name="psDN", bufs=1, space=PSUM) as psDN,
                tc.tile_pool(name="psCT", bufs=2, space=PSUM) as psCT,
                tc.tile_pool(name="psX", bufs=1, space=PSUM) as psX,
                tc.tile_pool(name="psO", bufs=1, space=PSUM) as psO,
            ):
                x_ps = None
                for g in range(N_GRP):
                    gsl = slice(g * GCOL, (g + 1) * GCOL)
                    sc_a = psSA.tile([128, GCOL], F32, tag="sc_a", name="sc_a")
                    sc_b = psSB.tile([8, GCOL], F32, tag="sc_b", name="sc_b")
                    nc.tensor.matmul(sc_a, ub_sb, rhs2_sb[:, gsl],
                                     start=True, stop=False)
                    nc.tensor.matmul(sc_b, ubb_sb[:, :], rhs2b_sb[:, gsl],
                                     start=True, stop=False)
                    for tt in range(GRP):
                        t = g * GRP + tt
                        csl = slice(tt * H * T, (tt + 1) * H * T)
                        nc.tensor.matmul(sc_a[:, csl], aT_cols(t, 128),
                                         qw_tile(t), start=False,
                                         stop=True, skip_group_check=True)
                        nc.tensor.matmul(sc_b[:, csl], aT_cols(t + 1, 8),
                                         qw_tile(t), start=False,
                                         stop=True, skip_group_check=True)
                    exp_a = ew.tile([128, GCOL], BF16, tag="exp_a",
                                    name="exp_a")
                    exp_b = ew.tile([8, GCOL], BF16, tag="exp_b", name="exp_b")
                    nc.scalar.activation(exp_a, sc_a, AFT.Exp)
                    nc.scalar.activation(exp_b, sc_b, AFT.Exp)
                    dnb = psDN.tile([128, GCOL], F32, tag="dnb", name="dnb")
                    nc.tensor.matmul(dnb, ones_a, exp_a,
                                     start=True, stop=False)
                    nc.tensor.matmul(dnb, ones_a[0:8, :], exp_b,
                                     start=False, stop=True,
                                     skip_group_check=True)
                    rec = ew.tile([128, GCOL], F32, tag="rec", name="rec")
                    nc.vector.reciprocal_approx_fast(rec, dnb)
                    p_a = ew.tile([128, GCOL], BF16, tag="p_a", name="p_a")
                    p_b = ew.tile([8, GCOL], BF16, tag="p_b", name="p_b")
                    nc.vector.tensor_tensor(p_a, exp_a, rec, AOP.mult)
                    nc.vector.tensor_tensor(p_b, exp_b, rec[0:8, :], AOP.mult)
                    ctx = psCT.tile([128, GCOL], F32, tag="ctx", name="ctx")
                    for tt in range(GRP):
                        t = g * GRP + tt
                        csl = slice(tt * H * T, (tt + 1) * H * T)
                        nc.tensor.matmul(ctx[:, csl], a_chunk(t),
                                         p_a[:, csl], start=True, stop=False,
                                         skip_group_check=True)
                        nc.tensor.matmul(ctx[:, csl], a_chunk(t + 1)[0:8, :],
                                         p_b[:, csl], start=False, stop=True,
                                         skip_group_check=True)
                    ctx_sb = ew.tile([128, GCOL], BF16, tag="ctx_sb",
                                     name="ctx_sb")
                    nc.vector.tensor_copy(ctx_sb, ctx)
                    if g % XG == 0:
                        x_ps = psX.tile([128, 512], F32, tag="x_ps",
                                        name="x_ps")
                    xo = (g % XG) * 128
                    for h in range(H):
                        nc.tensor.matmul(
                            x_ps[:, xo:xo + 128],
                            wv_sb[:, h, :],
                            ctx_sb[:, :]
                            .rearrange("p (t c) -> p t c", c=H * T)
                            [:, :, h * T:(h + 1) * T],
                            start=(h == 0), stop=(h == 3),
                            skip_group_check=True)
                    if g % XG == XG - 1:
                        sub = g // XG
                        ssl = slice(sub * 512, (sub + 1) * 512)
                        xb = ew.tile([128, 512], BF16, tag="xb", name="xb")
                        nc.vector.tensor_scalar(xb, x_ps, cv_sb, None, AOP.add)
                        nc.vector.tensor_tensor(xb, xb, gsig[sub], AOP.mult)
                        for c in range(4):
                            ps_o = psO.tile([128, 512], F32, tag="ps_o",
                                            name="ps_o")
                            nc.tensor.matmul(ps_o, wo_sb[:, c, :], xb,
                                             start=True, stop=True)
                            ot_sb = ew.tile([128, 512], F32, tag="ot_sb",
                                            name="ot_sb")
                            nc.scalar.activation(ot_sb, ps_o, AFT.Copy)
                            nc.sync.dma_start(o_t[c, :, ssl], ot_sb)
    nc.compile()
    nc.finalize()
    return nc


def _prep(s, a, starts, counts, token_mask, w_q, w_k, w_v, w_g, w_o,
          ln_q_g, ln_q_b, ln_kv_g, ln_kv_b):
    bf = ml_dtypes.bfloat16
    sc = 1.0 / np.sqrt(np.float32(D_H))
    wq1 = ((ln_q_g[:, None] * w_q) * sc).astype(bf)
    wg1 = (ln_q_g[:, None] * w_g).astype(bf)
    # head-masked weight blocks (avoid partition-offset matmul operands):
    # wk1t[k, h*128+m] = wk1.T[k, m] if k in head-h block else 0
    wk1_t = np.asarray((ln_kv_g[:, None] * w_k).T, np.float32)  # [dout, din]
    wk1t = np.zeros((128, 4 * 128), np.float32)
    wv1_f = np.asarray(ln_kv_g[:, None] * w_v, np.float32)      # [din, dd]
    wv1 = np.zeros((128, 4 * 128), np.float32)
    for h in range(4):
        wk1t[h * 32:(h + 1) * 32, h * 128:(h + 1) * 128] = \
            wk1_t[h * 32:(h + 1) * 32, :]
        wv1[:, h * 128:(h + 1) * 128] = wv1_f * \
            (np.arange(128)[None, :] // 32 == h)
    wk1t = wk1t.astype(bf)
    wv1 = wv1.astype(bf)
    cq = ((ln_q_b @ w_q) * sc).astype(np.float32).reshape(128, 1)
    cg = (ln_q_b @ w_g).astype(np.float32).reshape(128, 1)
    cv = (ln_kv_b @ w_v).astype(np.float32).reshape(128, 1)

    jj = np.arange(128)
    ub = (NEG * (jj[None, :] > np.arange(128)[:, None])).astype(np.float32)
    ub[127, :] = NEG
    ubias = ub.astype(bf)
    jb = np.arange(8)
    ubb = (NEG * (jb[None, :] > np.arange(9)[:, None])).astype(np.float32)
    ubb[8, :] = NEG
    ubiasb = ubb.astype(bf)

    shards = []
    A_need = 128 * TILES + SPAN_B
    for c in range(NC_CORES):
        b, half = c // 2, c % 2
        n0 = half * TOK
        st = np.asarray(starts[b, n0:n0 + TOK], np.int64)
        ct = np.asarray(counts[b, n0:n0 + TOK], np.int64)
        lo = int(st.min())
        st_loc = st - lo
        end_loc = st_loc + ct
        bases = 128 * (np.arange(TOK) // T)
        off = st_loc - bases
        end = end_loc - bases
        assert off.min() >= 0 and off.max() <= 127, \
            f"window premise violated (off {off.min()}..{off.max()})"
        assert end.max() <= 128 + SPAN_B, \
            f"window premise violated (end max {end.max()})"
        shards.append((b, n0, lo, off, end))
        A_need = max(A_need, int(end_loc.max()))
    A_pad = ((A_need + 127) // 128) * 128

    k_tok = np.arange(TOK)
    t_idx = k_tok // T
    i_idx = k_tok % T

    in_maps = []
    for (b, n0, lo, off, end) in shards:
        a_sl = np.zeros((A_pad, 128), np.float32)
        hi = min(lo + A_pad, M)
        a_sl[:hi - lo] = np.asarray(a[b, lo:hi, :], np.float32)
        # partition-major: [128 p, CH_A c, 128 d], atom (c*128+p)
        a8 = a_sl.reshape(A_pad // 128, 128, 128).transpose(1, 0, 2) \
            .reshape(128, A_pad).astype(bf)
        s_sl = np.asarray(s[b, n0:n0 + TOK, :], np.float32)
        s8 = s_sl.reshape(16, 128, 512).transpose(1, 0, 2) \
            .reshape(128, 16 * 512).astype(bf)

        r2 = np.zeros((128, COLS), np.float32)
        r2b = np.zeros((9, COLS), np.float32)
        for h in range(H):
            cols = t_idx * (H * T) + h * T + i_idx
            m1 = off >= 1
            np.add.at(r2, (np.where(m1, off - 1, 0), cols),
                      np.where(m1, -1.0, 0.0))
            np.add.at(r2, (np.full(TOK, 127), cols), np.where(m1, 1.0, 0.0))
            m2 = end <= 127
            np.add.at(r2, (np.where(m2, end - 1, 0), cols),
                      np.where(m2, 1.0, 0.0))
            m3 = end <= 128
            np.add.at(r2b, (np.full(TOK, 8), cols), np.where(m3, 1.0, 0.0))
            m4 = end >= 129
            np.add.at(r2b, (np.where(m4, end - 129, 0), cols),
                      np.where(m4, 1.0, 0.0))
        in_maps.append({
            "a8": a8, "s8": s8,
            "rhs2": r2.astype(bf), "rhs2b": r2b.astype(bf),
            "ubias": ubias, "ubiasb": ubiasb,
            "wq1": wq1, "wg1": wg1, "wk1t": wk1t, "wv1": wv1,
            "wo": np.asarray(w_o, np.float32).astype(bf),
            "cq": cq, "cg": cg, "cv": cv,
        })
    return in_maps, A_pad


def kernel(s, a, token_atom_starts, token_atom_counts, token_mask,
           w_q, w_k, w_v, w_g, w_o, ln_q_g, ln_q_b, ln_kv_g, ln_kv_b,
           trace=False):
    args = [np.asarray(x) for x in
            (s, a, token_atom_starts, token_atom_counts, token_mask,
             w_q, w_k, w_v, w_g, w_o, ln_q_g, ln_q_b, ln_kv_g, ln_kv_b)]
    in_maps, A_pad = _prep(*args)
    if A_pad not in _cache:
        nc = bacc.Bacc(None, target_bir_lowering=False)
        _cache[A_pad] = _build(nc, A_pad)
    nc = _cache[A_pad]
    res = run_bass_kernel_spmd(nc, in_maps, list(range(NC_CORES)),
                               trace=trace)
    out = np.zeros((B, N, D_TOK), np.float32)
    for c in range(NC_CORES):
        b, half = c // 2, c % 2
        n0 = half * TOK
        ot = res.results[c]["o_t"]          # [4, 128, TOK]
        tm = np.asarray(args[4][b, n0:n0 + TOK], np.float32)
        out[b, n0:n0 + TOK, :] = ot.reshape(512, TOK).T * tm[:, None]
    kernel.last_exec_time_ns = res.exec_time_ns
    return out
